# revision 27
# baseline (speedup 1.0000x reference)
"""Trainium2 Bass kernel for Llama GQA attention (B=2, S=2048, H=4096,
32 Q heads / 8 KV heads, head_dim 128, RoPE, causal).

Sharding: tensor-parallel by head across 8 cores. Core c owns Q heads
[4c..4c+3] and KV head c. Each core computes its Q/K/V projections,
RoPE, causal attention, and a partial output projection over its 512
attention features; the host sums the 8 partial outputs.

Device layout is feature-major ([feature, token]) throughout:
  - QKV proj:  Q'[f,t] (psum) = sum_h WqT[h,f].T @ xT[h,t]     (bf16)
  - RoPE:      q*cos + swap_halves(q)*sign*sin written straight into
               SBUF-resident q/k tiles (no DRAM round trip)
  - scores:    S.T[k,q] = K'[d,k].T @ Q'[d,q], two key-tiles per
               2-bank PSUM mega tile; one EXP per pair (ACT)
  - softmax:   denominator via ones-column matmul over the DVE pair-sum
               e0+e1 (halves the M=1 matmul stream), fast reciprocal,
               K=1 gpsimd broadcast, normalize fused into psum evict
  - AV:        U[d,q] = Vtok[k,d].T @ E[k,q]    (bf16, causal-sliced)
  - out:       out[t,o] = attn'[f,t].T @ WoT[f,o]  (partial; host sums)

Scheduling: weight/x DMAs are batched (4 contraction tiles per issue)
and emitted just-in-time so the PE starts within a few us.  O-proj
tiles drip into the attention stream of BOTH batches (qb-outer loop
makes attn ready early), keeping the PE dense end to end.
"""
import math
import numpy as np
import ml_dtypes

import concourse.bacc as bacc
import concourse.tile as tile
from concourse import mybir
from concourse.bass_utils import run_bass_kernel_spmd

F32 = mybir.dt.float32
BF16 = mybir.dt.bfloat16

P = 128
B, S, H = 2, 2048, 4096
T = B * S
DK = 128
NHL = 4                      # q heads per core
FL = NHL * DK                # 512 attention features per core
TB = 512                     # token block in phase 1
NTB = T // TB                # 8
NA = H // P                  # 32 contraction tiles
AG = 4                       # a-tiles per DMA group
NG = NA // AG                # 8
QBS = 512                    # query block
NQB = S // QBS               # 4
NKT = S // P                 # 16 key tiles per batch
SCALE = 1.0 / math.sqrt(DK)
NOB = H // 512               # 8 output column blocks
NTPB = S // P                # 16 output row tiles per batch

_NC_CACHE = {}


def build():
    nc = bacc.Bacc(None, target_bir_lowering=False)

    xt = nc.dram_tensor("xt", [H, T], BF16, kind="ExternalInput")
    wqt = nc.dram_tensor("wqt", [H, FL], BF16, kind="ExternalInput")
    wkt = nc.dram_tensor("wkt", [H, DK], BF16, kind="ExternalInput")
    wvt = nc.dram_tensor("wvt", [H, DK], BF16, kind="ExternalInput")
    wot = nc.dram_tensor("wot", [FL, H], BF16, kind="ExternalInput")
    cost = nc.dram_tensor("cost", [P, S], F32, kind="ExternalInput")
    sints = nc.dram_tensor("sints", [P, S], F32, kind="ExternalInput")
    trimask = nc.dram_tensor("trimask", [P, P], BF16, kind="ExternalInput")
    dmask = nc.dram_tensor("dmask", [P, 2 * P], BF16, kind="ExternalInput")
    onesc = nc.dram_tensor("onesc", [P, 1], BF16, kind="ExternalInput")
    out = nc.dram_tensor("out", [T, H], F32, kind="ExternalOutput")

    xt_v = xt.rearrange("(a p) t -> p a t", p=P)
    wq_v = wqt.rearrange("(a p) f -> p a f", p=P)
    wk_v = wkt.rearrange("(a p) f -> p a f", p=P)
    wv_v = wvt.rearrange("(a p) f -> p a f", p=P)
    wo_v = wot.rearrange("(j p) h -> p j h", p=P)

    EXP = mybir.ActivationFunctionType.Exp

    with nc.allow_low_precision(reason="attention compute dtypes are "
                                       "deliberately reduced"), \
         tile.TileContext(nc) as tc:
        with tc.tile_pool(name="const", bufs=1) as cp, \
             tc.tile_pool(name="res", bufs=1) as rsp, \
             tc.tile_pool(name="ep", bufs=4) as ep, \
             tc.tile_pool(name="pp", bufs=2) as ppool, \
             tc.tile_pool(name="rr", bufs=2) as rrp, \
             tc.tile_pool(name="p3o", bufs=6) as p3o, \
             tc.tile_pool(name="dram", bufs=1, space="DRAM") as dp:
            tri_sb = cp.tile([P, P], BF16)
            dm_sb = cp.tile([P, 2 * P], BF16)
            oc_sb = cp.tile([P, 1], BF16)
            warm_sb = cp.tile([P, 1], F32)
            # small constants ride the scalar (ACT) HWDGE ring so they
            # never delay the weight/x stream on the sync ring
            nc.scalar.dma_start(out=tri_sb, in_=trimask[:, :])
            nc.scalar.dma_start(out=dm_sb, in_=dmask[:, :])
            nc.scalar.dma_start(out=oc_sb, in_=onesc[:, :])
            # pre-warm the exp table set (~2.7us) while the PE ramps up so
            # the first real EXP doesn't pay the ACT_TABLE_LOAD mid-kernel
            nc.scalar.activation(warm_sb, oc_sb,
                                 mybir.ActivationFunctionType.Exp)

            q_res = [rsp.tile([P, S], BF16, name=f"q{b}_{h}")
                     for b in range(B) for h in range(NHL)]
            k_res = [rsp.tile([P, S], BF16, name=f"k{b}") for b in range(B)]
            vtk = [rsp.tile([P, NKT, P], BF16, name=f"vt{b}")
                   for b in range(B)]
            v_scr = [dp.tile([DK, S], BF16, name=f"vscr{b}")
                     for b in range(B)]

            # ---------------- Phase 1: QKV projection + RoPE ----------------
            with tc.tile_pool(name="w1", bufs=1) as wp, \
                 tc.tile_pool(name="cs", bufs=1) as csp, \
                 tc.tile_pool(name="xp", bufs=2) as xp, \
                 tc.tile_pool(name="rp", bufs=1) as rp, \
                 tc.tile_pool(name="ps1", bufs=1, space="PSUM") as ps1:
                wq_sb = wp.tile([P, NA, FL], BF16)
                wk_sb = wp.tile([P, NA, DK], BF16)
                wv_sb = wp.tile([P, NA, DK], BF16)
                cos_sb = csp.tile([P, S], F32)
                sin_sb = csp.tile([P, S], F32)

                for tb in range(NTB):
                    bi = (tb * TB) // S
                    s0 = (tb * TB) % S
                    if tb == NTB // 2:
                        # batch-0 V complete: transpose now so vtk[0] is
                        # long ready when attention starts
                        nc.sync.dma_start_transpose(vtk[0], v_scr[0][:, :])
                    psq = [ps1.tile([P, TB], F32, name=f"psq{j}_{tb}",
                                    tag=f"psq{j}") for j in range(NHL)]
                    psk = ps1.tile([P, TB], F32, name=f"psk_{tb}", tag="psk")
                    psv = ps1.tile([P, TB], F32, name=f"psv_{tb}", tag="psv")
                    gsz = AG if tb == 0 else 2 * AG
                    for g in range(NA // gsz):
                        ga, gb = g * gsz, (g + 1) * gsz
                        xt_t = xp.tile([P, gsz, TB], BF16, name=f"x_{tb}_{g}",
                                       tag="xt")
                        if tb == 0 and g == 0:
                            # per-a loads so the very first matmul waits on
                            # one 128-row tile; weights ride the (idle)
                            # scalar ring so the two issue streams overlap
                            for a in range(ga, gb):
                                nc.scalar.dma_start(out=wk_sb[:, a:a + 1, :],
                                                    in_=wk_v[:, a:a + 1, :])
                                nc.sync.dma_start(
                                    out=xt_t[:, a - ga:a - ga + 1, :],
                                    in_=xt_v[:, a:a + 1, 0:TB])
                                nc.scalar.dma_start(out=wv_sb[:, a:a + 1, :],
                                                    in_=wv_v[:, a:a + 1, :])
                                nc.scalar.dma_start(out=wq_sb[:, a:a + 1, :],
                                                    in_=wq_v[:, a:a + 1, :])
                        else:
                            if tb == 0:
                                nc.scalar.dma_start(out=wk_sb[:, ga:gb, :],
                                                    in_=wk_v[:, ga:gb, :])
                                nc.scalar.dma_start(out=wv_sb[:, ga:gb, :],
                                                    in_=wv_v[:, ga:gb, :])
                                nc.scalar.dma_start(out=wq_sb[:, ga:gb, :],
                                                    in_=wq_v[:, ga:gb, :])
                            nc.sync.dma_start(
                                out=xt_t,
                                in_=xt_v[:, ga:gb, tb * TB:(tb + 1) * TB])
                        if tb == 0 and g == 1:
                            nc.sync.dma_start(out=cos_sb, in_=cost[:, :])
                            nc.sync.dma_start(out=sin_sb, in_=sints[:, :])
                        for j in range(gsz):
                            a = ga + j
                            st, sp = (a == 0), (a == NA - 1)
                            nc.tensor.matmul(psk, wk_sb[:, a, :],
                                             xt_t[:, j, :], start=st, stop=sp)
                            nc.tensor.matmul(psv, wv_sb[:, a, :],
                                             xt_t[:, j, :], start=st, stop=sp)
                            for jq in range(NHL):
                                nc.tensor.matmul(
                                    psq[jq],
                                    wq_sb[:, a, jq * DK:(jq + 1) * DK],
                                    xt_t[:, j, :], start=st, stop=sp)

                    # evict psum banks (one reader each, split ACT/DVE),
                    # RoPE in SBUF, result written straight into the
                    # resident q/k tiles
                    plan = [(psk, k_res[bi], nc.scalar),
                            (psq[0], q_res[bi * NHL + 0], nc.vector),
                            (psq[1], q_res[bi * NHL + 1], nc.scalar),
                            (psq[2], q_res[bi * NHL + 2], nc.vector),
                            (psq[3], q_res[bi * NHL + 3], nc.scalar)]
                    for idx, (src, dest, eng) in enumerate(plan):
                        qc = rp.tile([P, TB], F32, name=f"qc_{tb}_{idx}",
                                     tag="qc", bufs=8)
                        if eng is nc.scalar:
                            nc.scalar.copy(qc, src)
                        else:
                            nc.vector.tensor_copy(qc, src)
                        if idx == 0:
                            vb = rp.tile([P, TB], BF16, name=f"vb_{tb}",
                                         tag="vb", bufs=2)
                            nc.vector.tensor_copy(vb, psv)
                            nc.sync.dma_start(
                                out=v_scr[bi][:, s0:s0 + TB], in_=vb)
                        # swaps ride the sync ring: the scalar ring must
                        # stay clear so the first attention EXPs aren't
                        # queued behind DMA issues at the phase boundary
                        sw = rp.tile([P, TB], F32, name=f"sw_{tb}_{idx}",
                                     tag="sw", bufs=8)
                        nc.sync.dma_start(out=sw[0:64, :], in_=qc[64:128, :])
                        nc.sync.dma_start(out=sw[64:128, :], in_=qc[0:64, :])
                        nc.vector.tensor_mul(qc, qc, cos_sb[:, s0:s0 + TB])
                        nc.vector.tensor_mul(sw, sw, sin_sb[:, s0:s0 + TB])
                        nc.vector.tensor_add(dest[:, s0:s0 + TB], qc, sw)

            # ------------- Phase 2: attention + output projection ----------
            with tc.tile_pool(name="wo", bufs=1) as wop, \
                 tc.tile_pool(name="attn", bufs=1) as ap2, \
                 tc.tile_pool(name="ps_s", bufs=1, space="PSUM") as ps_s, \
                 tc.tile_pool(name="ps_u", bufs=2, space="PSUM") as ps_u, \
                 tc.tile_pool(name="ps_d", bufs=2, space="PSUM") as ps_d, \
                 tc.tile_pool(name="ps_o", bufs=2, space="PSUM") as ps_o:
                wo_sb = [wop.tile([P, NHL, 512], BF16, name=f"wo{ob}")
                         for ob in range(NOB)]
                for ob in range(NOB):
                    nc.sync.dma_start(out=wo_sb[ob],
                                      in_=wo_v[:, :, ob * 512:(ob + 1) * 512])
                attn_sb = [[ap2.tile([P, S], BF16, name=f"at{b}_{h}")
                            for h in range(NHL)] for b in range(B)]

                avail = []
                cursor = [0]
                ocnt = [0]

                def emit_otile(bt, ob, ti):
                    o_ps = ps_o.tile([P, 512], F32, name=f"o_{ocnt[0]}",
                                     tag="o")
                    for j in range(NHL):
                        nc.tensor.matmul(
                            o_ps, attn_sb[bt][j][:, ti * P:(ti + 1) * P],
                            wo_sb[ob][:, j, :],
                            start=(j == 0), stop=(j == NHL - 1))
                    o_sb = p3o.tile([P, 512], F32, name=f"os_{ocnt[0]}",
                                    tag="os")
                    if ocnt[0] % 2 == 0:
                        nc.scalar.copy(o_sb, o_ps)
                    else:
                        nc.vector.tensor_copy(o_sb, o_ps)
                    ocnt[0] += 1
                    tt = bt * NTPB + ti
                    nc.sync.dma_start(
                        out=out[tt * P:(tt + 1) * P, ob * 512:(ob + 1) * 512],
                        in_=o_sb)

                def drip(k):
                    n = 0
                    while cursor[0] < len(avail) and n < k:
                        emit_otile(*avail[cursor[0]])
                        cursor[0] += 1
                        n += 1

                # deferred denominator + normalize for the previous unit;
                # flushed after the next unit's first score pair so the d
                # matmuls never make the in-order PE queue wait on the DVE
                pending = [None]

                def flush_pending():
                    if pending[0] is not None:
                        pending[0]()
                        pending[0] = None

                for b in range(B):
                    if b == 1:
                        # deferred so the xbar-mode serialization against
                        # the phase-1 SBUF->SBUF RoPE-swap DMAs doesn't
                        # stall the first attention unit
                        nc.sync.dma_start_transpose(vtk[1], v_scr[1][:, :])
                    # qb=1 first: its leading pairs are mask-free, so the
                    # PE isn't stuck behind the phase-1 RoPE backlog on DVE
                    for qb in (1, 0, 2, 3):
                        for h in range(NHL):
                            qh = q_res[b * NHL + h]
                            npair = 2 * (qb + 1)
                            u_ps = ps_u.tile([P, QBS], F32,
                                             name=f"u_{b}_{h}_{qb}", tag="u")
                            d_ps = ps_d.tile([1, QBS], F32,
                                             name=f"d_{b}_{h}_{qb}", tag="d")
                            fifo = []
                            dlist = []
                            pend_full = []
                            pend_diag = []

                            def flush_av(fifo=fifo, u_ps=u_ps,
                                         npair=npair, b=b):
                                pr0, e_t, lo0, lo1 = fifo.pop(0)
                                kt0, kt1 = 2 * pr0, 2 * pr0 + 1
                                st, sp = (pr0 == 0), (pr0 == npair - 1)
                                nc.tensor.matmul(
                                    u_ps[:, lo0:], vtk[b][:, kt0, :],
                                    e_t[:, lo0:QBS], start=st, stop=False,
                                    skip_group_check=True)
                                nc.tensor.matmul(
                                    u_ps[:, lo1:], vtk[b][:, kt1, :],
                                    e_t[:, QBS + lo1:], start=False, stop=sp,
                                    skip_group_check=True)

                            for pr in range(npair):
                                kt0, kt1 = 2 * pr, 2 * pr + 1
                                m0 = kt0 - 4 * qb
                                lo0 = m0 * P if m0 > 0 else 0
                                lo1 = (m0 + 1) * P if m0 + 1 > 0 else 0
                                sm = ps_s.tile([P, 2 * QBS], F32,
                                               name=f"s_{b}_{h}_{qb}_{pr}",
                                               tag="sm")
                                nc.tensor.matmul(
                                    sm[:, 0:QBS],
                                    k_res[b][:, kt0 * P:(kt0 + 1) * P],
                                    qh[:, qb * QBS:(qb + 1) * QBS],
                                    start=True, stop=True)
                                nc.tensor.matmul(
                                    sm[:, QBS:],
                                    k_res[b][:, kt1 * P:(kt1 + 1) * P],
                                    qh[:, qb * QBS:(qb + 1) * QBS],
                                    start=True, stop=True)
                                e_t = ep.tile([P, 2 * QBS], BF16,
                                              name=f"e_{b}_{h}_{qb}_{pr}",
                                              tag="e")
                                nc.scalar.activation(e_t, sm, EXP,
                                                     scale=SCALE)
                                if pr == 0:
                                    flush_pending()
                                if m0 >= 0:
                                    # diagonal pair: causal masks
                                    nc.vector.tensor_mul(
                                        e_t[:, lo0:lo0 + P],
                                        e_t[:, lo0:lo0 + P], tri_sb)
                                    nc.vector.tensor_mul(
                                        e_t[:, QBS + lo0:QBS + lo0 + 2 * P],
                                        e_t[:, QBS + lo0:QBS + lo0 + 2 * P],
                                        dm_sb)
                                ps_t = ppool.tile([P, QBS], BF16,
                                                  name=f"p_{b}_{h}_{qb}_{pr}",
                                                  tag="ps", bufs=9)
                                nc.vector.tensor_add(ps_t[:, lo0:],
                                                     e_t[:, lo0:QBS],
                                                     e_t[:, QBS + lo0:])
                                if m0 >= 0:
                                    if pend_full:
                                        dlist.append((pend_full.pop(), 0))
                                    # merge the unit's two diagonal
                                    # pair-sums with one in-place add
                                    if pend_diag:
                                        pd = pend_diag.pop()
                                        nc.vector.tensor_add(
                                            pd[:, lo0:], pd[:, lo0:],
                                            ps_t[:, lo0:])
                                        dlist.append((pd, 0))
                                    else:
                                        pend_diag.append(ps_t)
                                elif pend_full:
                                    # fold two full pair-sums into one so
                                    # the denominator matmul streams half
                                    # as many columns through the PE
                                    pq = ppool.tile(
                                        [P, QBS], BF16,
                                        name=f"q_{b}_{h}_{qb}_{pr}",
                                        tag="psq2", bufs=5)
                                    nc.vector.tensor_add(
                                        pq, pend_full.pop(), ps_t)
                                    dlist.append((pq, 0))
                                else:
                                    pend_full.append(ps_t)
                                if len(fifo) >= 1:
                                    flush_av()
                                fifo.append((pr, e_t, lo0, lo1))
                                # qb0/qb1 units have little attention PE
                                # work per exp; feed them more o-proj
                                # tiles, and hold tiles back in qb3 so the
                                # next batch's qb0/qb1 have a reserve
                                drip(2 if qb <= 1 else 1)
                            while fifo:
                                flush_av()
                            if pend_full:
                                dlist.append((pend_full.pop(), 0))
                            if pend_diag:
                                dlist.append((pend_diag.pop(), 0))

                            def finish(dlist=dlist, u_ps=u_ps, d_ps=d_ps,
                                       b=b, h=h, qb=qb):
                                n = len(dlist)
                                for i, (ps_t, lo0) in enumerate(dlist):
                                    nc.tensor.matmul(
                                        d_ps[:, lo0:], oc_sb, ps_t[:, lo0:],
                                        start=(i == 0), stop=(i == n - 1),
                                        skip_group_check=True)
                                rf = rrp.tile([1, QBS], F32,
                                              name=f"rf_{b}_{h}_{qb}",
                                              tag="rf")
                                nc.vector.reciprocal_approx_fast(rf, d_ps)
                                rs = rrp.tile([P, QBS], F32,
                                              name=f"rs_{b}_{h}_{qb}",
                                              tag="rs")
                                nc.gpsimd.partition_broadcast(rs, rf)
                                nc.vector.tensor_mul(
                                    attn_sb[b][h][:, qb * QBS:(qb + 1) * QBS],
                                    u_ps, rs)
                            pending[0] = finish
                        flush_pending()
                        avail.extend((b, ob, 4 * qb + kk)
                                     for ob in range(NOB) for kk in range(4))
                while cursor[0] < len(avail):
                    emit_otile(*avail[cursor[0]])
                    cursor[0] += 1

    nc.compile()
    return nc


def _prep_inputs(hidden_states, Wq, Wk, Wv, Wo, cos, sin):
    hs = np.asarray(hidden_states, dtype=np.float32)
    Wq = np.asarray(Wq, dtype=np.float32)
    Wk = np.asarray(Wk, dtype=np.float32)
    Wv = np.asarray(Wv, dtype=np.float32)
    Wo = np.asarray(Wo, dtype=np.float32)
    cos = np.asarray(cos, dtype=np.float32)
    sin = np.asarray(sin, dtype=np.float32)

    xt = np.ascontiguousarray(hs.reshape(T, H).T).astype(ml_dtypes.bfloat16)
    cosT = np.ascontiguousarray(cos.T)
    sinT = np.ascontiguousarray(sin.T)
    sints = np.ascontiguousarray(
        np.concatenate([-sinT[:64], sinT[64:]], axis=0))
    kq = np.arange(P)
    trim = (kq[None, :] >= kq[:, None]).astype(ml_dtypes.bfloat16)
    dmask = np.concatenate(
        [np.zeros((P, P), dtype=ml_dtypes.bfloat16), trim], axis=1)
    onesc = np.ones((P, 1), dtype=ml_dtypes.bfloat16)

    in_maps = []
    for c in range(8):
        in_maps.append({
            "xt": xt,
            "wqt": np.ascontiguousarray(
                Wq[c * FL:(c + 1) * FL, :].T).astype(ml_dtypes.bfloat16),
            "wkt": np.ascontiguousarray(
                Wk[c * DK:(c + 1) * DK, :].T).astype(ml_dtypes.bfloat16),
            "wvt": np.ascontiguousarray(
                Wv[c * DK:(c + 1) * DK, :].T).astype(ml_dtypes.bfloat16),
            "wot": np.ascontiguousarray(
                Wo[:, c * FL:(c + 1) * FL].T).astype(ml_dtypes.bfloat16),
            "cost": cosT,
            "sints": sints,
            "trimask": trim,
            "dmask": dmask,
            "onesc": onesc,
        })
    return in_maps


def kernel(hidden_states, Wq, Wk, Wv, Wo, cos, sin, _run_kwargs=None):
    in_maps = _prep_inputs(hidden_states, Wq, Wk, Wv, Wo, cos, sin)
    if "nc" not in _NC_CACHE:
        _NC_CACHE["nc"] = build()
    nc = _NC_CACHE["nc"]
    kw = _run_kwargs or {}
    res = run_bass_kernel_spmd(nc, in_maps, core_ids=list(range(8)), **kw)
    acc = np.zeros((T, H), dtype=np.float64)
    for c in range(8):
        acc += np.asarray(res.results[c]["out"], dtype=np.float64)
    out = acc.astype(np.float32).reshape(B, S, H)
    if kw:
        _NC_CACHE["last_results"] = res
    return out
